# revision 34
# baseline (speedup 1.0000x reference)
"""Trainium2 Bass kernel for nn_Encoder_66065186947370 (3-level neighbor-attention encoder).

Sharding: B(2) x H-rows(4) = 8 cores. Each core computes its 32 H-rows of the
output plus a forward halo (circular window of 8 => 7-token halo per level,
taken at full-grid-row granularity), so no inter-core communication is needed.

On-chip layout is feature-major ([channel, token]); neighbor attention is done
in 120-token blocks as dense PE matmuls against a 127-wide position window with
a multiplicative exp-mask (band structure + exp(rel_bias/sqrt(C)) folded in,
off-band exactly 0). Algebraic folds (host-side, weights only):
  Wqk = wq @ wk.T  (scores = (x Wqk + wk bq) . x  up to softmax-invariant terms)
  Wvo = wv @ wo,  bvo = bv @ wo + bo   (attn = (P.X) Wvo + bvo, since P rows sum to 1)
"""

import sys

sys.path.insert(0, "/opt/trn_rl_repo")

import numpy as np

import concourse.bass as bass
import concourse.tile as tile
from concourse import bacc, mybir
from concourse.masks import make_identity

F32 = mybir.dt.float32
F32R = mybir.dt.float32r
BF16 = mybir.dt.bfloat16


def _r(ap):
    """Bitcast an fp32 AP to float32r for single-pass PE streaming (fast for
    moving dim >= 256; products rounded TF32-style on HW)."""
    return ap.bitcast(F32R)
AF = mybir.ActivationFunctionType
ALU = mybir.AluOpType

# ---- static problem geometry -------------------------------------------------
B, GRID, C0 = 2, 128, 128
NLEV = 3
K = 8  # window
TB = 120  # attention block tokens
WD = TB + K - 1  # 127 position window

D = [128, 256, 512]  # channels per level
W = [128, 64, 32]  # grid width per level
ROWS_IN = [39, 19, 9]  # grid rows of level input held per core (own + halo)
ROWS_NA = [38, 18, 8]  # grid rows of NA output computed per core
T_IN = [ROWS_IN[l] * W[l] for l in range(3)]  # 4992, 1216, 288
T_NA = [ROWS_NA[l] * W[l] for l in range(3)]  # 4864, 1152, 256
T_SKIP = [4096, 1024, 256]  # own NA tokens stored as skip output
T_MG = [1216, 288, 64]  # merge output tokens per core
OWN_ROWS = 32  # own L0 grid rows per core


def _ceil_div(a, b):
    return (a + b - 1) // b


def _build_program():
    nc = bacc.Bacc(None, target_bir_lowering=False)

    # ---- DRAM I/O ----
    x_d = nc.dram_tensor("x", [T_IN[0], C0], F32, kind="ExternalInput")
    wqk_d, wvo_d, bqk_d, bvo_d, mex_d, wm_d = [], [], [], [], [], []
    for l in range(NLEV):
        d = D[l]
        wqk_d.append(nc.dram_tensor(f"wqk{l}", [d, d], F32R, kind="ExternalInput"))
        wvo_d.append(nc.dram_tensor(f"wvo{l}", [d, d], F32R, kind="ExternalInput"))
        bqk_d.append(nc.dram_tensor(f"bqk{l}", [d], F32, kind="ExternalInput"))
        bvo_d.append(nc.dram_tensor(f"bvo{l}", [d], F32, kind="ExternalInput"))
        mex_d.append(nc.dram_tensor(f"mexp{l}", [TB, 4 * WD], F32, kind="ExternalInput"))
        wm_d.append(
            nc.dram_tensor(
                f"wm{l}", [4 * d, 2 * d], BF16 if l == 2 else F32R,
                kind="ExternalInput",
            )
        )
    bm_d = [
        nc.dram_tensor("bm0", [2 * D[0]], F32, kind="ExternalInput"),
        nc.dram_tensor("bm1", [128, 2 * D[1]], F32, kind="ExternalInput"),
        nc.dram_tensor("bm2", [T_MG[2], 2 * D[2]], F32, kind="ExternalInput"),
    ]
    skip_d = [
        nc.dram_tensor(f"skip{l}", [D[l], T_SKIP[l]], F32, kind="ExternalOutput")
        for l in range(NLEV)
    ]
    fin_d = nc.dram_tensor("fin", [T_MG[2], 2 * D[2]], F32, kind="ExternalOutput")

    with tile.TileContext(nc) as tc:
        with (
            tc.tile_pool(name="const", bufs=1) as constp,
            tc.tile_pool(name="wts", bufs=1) as wtsp,
            tc.tile_pool(name="big", bufs=1) as bigp,
            tc.tile_pool(name="wm2s", bufs=12) as wm2p,
            tc.tile_pool(name="blk", bufs=3) as blkp,
            tc.tile_pool(name="ps", bufs=2, space="PSUM") as psp,
            tc.tile_pool(name="pst", bufs=2, space="PSUM") as pstp,
        ):
            ident = constp.tile([128, 128], F32)
            make_identity(nc, ident[:])
            identb = constp.tile([128, 128], BF16)
            make_identity(nc, identb[:])

            # ---- load weights/constants (wm2 streamed later) ----
            wqk_s, wvo_s, bqk_s, bvo_s, mex_s, wm_s = [], [], [], [], [], []
            for l in range(NLEV):
                d = D[l]
                nct = d // 128
                wq = wtsp.tile([128, nct * d], F32R, tag=f"wqk{l}")
                wv = wtsp.tile([128, nct * d], F32R, tag=f"wvo{l}")
                nc.gpsimd.dma_start(
                    out=wq[:].rearrange("p (k dd) -> p k dd", k=nct),
                    in_=wqk_d[l][:].rearrange("(k p) dd -> p k dd", p=128),
                )
                nc.gpsimd.dma_start(
                    out=wv[:].rearrange("p (k dd) -> p k dd", k=nct),
                    in_=wvo_d[l][:].rearrange("(k p) dd -> p k dd", p=128),
                )
                bq = wtsp.tile([128, nct], F32, tag=f"bqk{l}")
                bv = wtsp.tile([128, nct], F32, tag=f"bvo{l}")
                nc.gpsimd.dma_start(
                    out=bq[:], in_=bqk_d[l][:].rearrange("(k p) -> p k", p=128)
                )
                nc.gpsimd.dma_start(
                    out=bv[:], in_=bvo_d[l][:].rearrange("(k p) -> p k", p=128)
                )
                me = wtsp.tile([TB, 4 * WD], F32, tag=f"mexp{l}")
                nc.gpsimd.dma_start(out=me[:], in_=mex_d[l][:])
                if l < 2:
                    d2 = 2 * d
                    nk = 4 * d // 128
                    wm = wtsp.tile([128, nk * d2], F32R, tag=f"wm{l}")
                    nc.gpsimd.dma_start(
                        out=wm[:].rearrange("p (k d2) -> p k d2", k=nk),
                        in_=wm_d[l][:].rearrange("(k p) d2 -> p k d2", p=128),
                    )
                else:
                    wm = None
                wqk_s.append(wq)
                wvo_s.append(wv)
                bqk_s.append(bq)
                bvo_s.append(bv)
                mex_s.append(me)
                wm_s.append(wm)

            # ---- level-0 input: load token-major, PE-transpose to feature-major
            xT = [bigp.tile([128, T_IN[0]], F32, tag="xT_0")]
            for t in range(ROWS_IN[0]):
                xt = blkp.tile([128, 128], F32, tag="xin")
                nc.sync.dma_start(out=xt[:], in_=x_d[t * 128 : (t + 1) * 128, :])
                tp = pstp.tile([128, 128], F32, tag="tr")
                nc.tensor.transpose(tp[:], xt[:], ident[:])
                nc.any.tensor_copy(xT[0][:, t * 128 : (t + 1) * 128], tp[:])

            for l in range(NLEV):
                d = D[l]
                nct = d // 128
                t_in, t_na = T_IN[l], T_NA[l]
                inv_sqrt = 1.0 / float(np.sqrt(d))

                # ---- phase A: Q'^T = Wqk^T . x^T (+bias), feature-major
                qT = [
                    bigp.tile([128, t_na], F32, tag=f"big1_{ci}") for ci in range(nct)
                ]
                for cj in range(nct):
                    for t0 in range(0, t_na, 512):
                        n = min(512, t_na - t0)
                        ps = psp.tile([128, 512], F32, tag="mm")
                        for ci in range(nct):
                            nc.tensor.matmul(
                                ps[:, :n],
                                wqk_s[l][:, ci * d + cj * 128 : ci * d + cj * 128 + 128],
                                xT[ci][:, t0 : t0 + n],
                                start=(ci == 0),
                                stop=(ci == nct - 1),
                            )
                        nc.scalar.activation(
                            qT[cj][:, t0 : t0 + n],
                            ps[:, :n],
                            AF.Identity,
                            bias=bqk_s[l][:, cj : cj + 1],
                        )

                # ---- phase B: neighbor attention blocks -> Y^T feature-major
                yT = [
                    bigp.tile([128, t_na], F32, tag=f"big2_{ci}") for ci in range(nct)
                ]
                # attention runs in groups of up to 4 full blocks so the
                # softmax-side DVE/ACT ops work on wide tiles (one PSUM bank
                # holds 4 blocks of scores); the ragged tail block runs alone.
                nfull = t_na // TB
                groups = [
                    (g0, min(4, nfull - g0)) for g0 in range(0, nfull, 4)
                ]
                if t_na % TB:
                    groups.append((nfull, 0))  # tail marker
                for g0, gn in groups:
                    if gn == 0:
                        gn, m, w = 1, t_na - g0 * TB, min(WD, t_in - g0 * TB)
                    else:
                        m, w = TB, WD
                    gw, gm = gn * WD, gn * TB
                    sc = psp.tile([TB, 4 * WD], F32, tag="mm")
                    for j in range(gn):
                        tb = (g0 + j) * TB
                        for ci in range(nct):
                            nc.tensor.matmul(
                                sc[:m, j * WD : j * WD + w],
                                qT[ci][:, tb : tb + m],
                                xT[ci][:, tb : tb + w].bitcast(F32),
                                start=(ci == 0),
                                stop=(ci == nct - 1),
                            )
                    # column range actually populated (tail: w < WD)
                    cw = (gn - 1) * WD + w
                    ex = blkp.tile([TB, 4 * WD], F32, tag="ex")
                    nc.scalar.activation(
                        ex[:m, :cw], sc[:m, :cw], AF.Exp, scale=inv_sqrt
                    )
                    en = blkp.tile([TB, 4 * WD], F32, tag="en")
                    # NB: fused tensor_tensor_reduce(accum_out=...) wedges the
                    # device on this walrus version; use two DVE ops instead.
                    nc.vector.tensor_mul(en[:m, :cw], ex[:m, :cw], mex_s[l][:m, :cw])
                    rs = blkp.tile([TB, 4], F32, tag="rs")
                    nc.vector.tensor_reduce(
                        rs[:m, :gn],
                        en[:m, :cw].rearrange("p (g w) -> p g w", g=gn)
                        if w == WD
                        else en[:m, :cw].unsqueeze(1),
                        axis=mybir.AxisListType.X,
                        op=ALU.add,
                    )
                    ri = blkp.tile([TB, 4], F32, tag="ri")
                    nc.vector.reciprocal(ri[:m, :gn], rs[:m, :gn])
                    ep = blkp.tile([TB, 4 * WD], BF16, tag="ep")
                    nc.vector.tensor_tensor(
                        ep[:m, :cw].rearrange("p (g w) -> p g w", g=gn)
                        if w == WD
                        else ep[:m, :cw].unsqueeze(1),
                        en[:m, :cw].rearrange("p (g w) -> p g w", g=gn)
                        if w == WD
                        else en[:m, :cw].unsqueeze(1),
                        ri[:m, :gn].unsqueeze(2).broadcast_to((m, gn, w)),
                        ALU.mult,
                    )
                    # transpose normalized probs per block: [m, w] -> [w, m]
                    et_ps = pstp.tile([WD, 4 * TB], BF16, tag="trb", name="et_ps")
                    for j in range(gn):
                        nc.tensor.transpose(
                            et_ps[:w, j * TB : j * TB + m],
                            ep[:m, j * WD : j * WD + w],
                            identb[:m, :m],
                        )
                    et = blkp.tile([WD, 4 * TB], BF16, tag="et")
                    nc.any.tensor_copy(et[:w, : (gn - 1) * TB + m], et_ps[:w, : (gn - 1) * TB + m])
                    for ci in range(nct):
                        xw_ps = pstp.tile([WD, 4 * 128], F32, tag="tr")
                        for j in range(gn):
                            tb = (g0 + j) * TB
                            nc.tensor.transpose(
                                xw_ps[:w, j * 128 : j * 128 + 128],
                                xT[ci][:, tb : tb + w].bitcast(F32),
                                ident[:],
                            )
                        xw = blkp.tile([WD, 4 * 128], BF16, tag="xw")
                        nc.any.tensor_copy(
                            xw[:w, : gn * 128], xw_ps[:w, : gn * 128]
                        )
                        yp = psp.tile([128, 4 * TB], F32, tag="yp")
                        for j in range(gn):
                            nc.tensor.matmul(
                                yp[:, j * TB : j * TB + m],
                                xw[:w, j * 128 : j * 128 + 128],
                                et[:w, j * TB : j * TB + m],
                                start=True,
                                stop=True,
                            )
                        nc.any.tensor_copy(
                            yT[ci][:, g0 * TB : g0 * TB + (gn - 1) * TB + m],
                            yp[:, : (gn - 1) * TB + m],
                        )

                # ---- phase C: attn^T = Wvo^T . Y^T (+bvo), feature-major
                aT = [
                    bigp.tile([128, t_na], F32, tag=f"big1_{ci}") for ci in range(nct)
                ]
                for cj in range(nct):
                    for t0 in range(0, t_na, 512):
                        n = min(512, t_na - t0)
                        ps = psp.tile([128, 512], F32, tag="mm")
                        for ci in range(nct):
                            nc.tensor.matmul(
                                ps[:, :n],
                                wvo_s[l][:, ci * d + cj * 128 : ci * d + cj * 128 + 128],
                                yT[ci][:, t0 : t0 + n],
                                start=(ci == 0),
                                stop=(ci == nct - 1),
                            )
                        if (t0 // 512) % 2 == 1:
                            nc.scalar.activation(
                                aT[cj][:, t0 : t0 + n],
                                ps[:, :n],
                                AF.Identity,
                                bias=bvo_s[l][:, cj : cj + 1],
                            )
                        else:
                            nc.vector.tensor_scalar_add(
                                aT[cj][:, t0 : t0 + n],
                                ps[:, :n],
                                bvo_s[l][:, cj : cj + 1],
                            )

                # ---- phase D: skip output, stored feature-major (host transposes)
                for cj in range(nct):
                    nc.sync.dma_start(
                        out=skip_d[l][cj * 128 : (cj + 1) * 128, :],
                        in_=aT[cj][:, : T_SKIP[l]].bitcast(F32),
                    )

                # ---- phase E: patch merge
                d2 = 2 * d
                rows_na = ROWS_NA[l]
                wl1 = W[l] // 2  # next-level grid width
                # group g reads (2h+a, 2w+b); channel row in w_merge is g*d + c
                gview = [
                    [
                        yT[ci].rearrange("c (h w) -> c h w", h=rows_na)[:, a::2, b::2]
                        for ci in range(nct)
                    ]
                    for (a, b) in ((0, 0), (1, 0), (0, 1), (1, 1))
                ]
                if l == 0:
                    # feature-major output x1T [2][128, T_MG[0]]
                    nxt = [
                        bigp.tile([128, T_MG[0]], F32, tag=f"xT_{cj}")
                        for cj in range(2)
                    ]
                    nrows = T_MG[0] // wl1  # 19
                    for cj in range(2):
                        for r0 in range(0, nrows, 8):
                            nr = min(8, nrows - r0)
                            n = nr * wl1
                            ps = psp.tile([128, 512], F32, tag="mm")
                            for g in range(4):
                                nc.tensor.matmul(
                                    ps[:, :n],
                                    wm_s[0][
                                        :, g * d2 + cj * 128 : g * d2 + cj * 128 + 128
                                    ],
                                    gview[g][0][:, r0 : r0 + nr, :],
                                    start=(g == 0),
                                    stop=(g == 3),
                                )
                            nc.scalar.activation(
                                nxt[cj][:, r0 * wl1 : r0 * wl1 + n],
                                ps[:, :n],
                                AF.Identity,
                                bias=bm0_s[:, cj : cj + 1],
                            )
                    xT = nxt
                elif l == 1:
                    # token-major output x2tok [3][<=128, 512]
                    x2tok = [
                        bigp.tile([128, d2], F32, tag=f"big2_{mi}") for mi in range(3)
                    ]
                    nk = 4 * d // 128  # 8
                    for mi, (r0, nr) in enumerate(((0, 4), (4, 4), (8, 1))):
                        mtok = nr * wl1
                        ps = psp.tile([128, 512], F32, tag="mm")
                        for k in range(nk):
                            g, ci = divmod(k, nct)
                            # stationary operand must have a single free dim:
                            # stage the strided view through a contiguous tile
                            cg = blkp.tile([128, 128], F32R, tag="mgl", name="mgl")
                            nc.vector.tensor_copy(
                                cg[:, :mtok].rearrange("c (r w) -> c r w", r=nr),
                                gview[g][ci][:, r0 : r0 + nr, :],
                            )
                            nc.tensor.matmul(
                                ps[:mtok, :],
                                cg[:, :mtok],
                                wm_s[1][:, k * d2 : (k + 1) * d2],
                                start=(k == 0),
                                stop=(k == nk - 1),
                            )
                        nc.vector.tensor_add(x2tok[mi][:mtok, :], ps[:mtok, :], bm1_s[:mtok, :])
                    # transpose to feature-major xT2 [4][128, 288]
                    nxt = [
                        bigp.tile([128, T_IN[2]], F32, tag=f"xT_{cj}")
                        for cj in range(4)
                    ]
                    for mi, (r0, nr) in enumerate(((0, 4), (4, 4), (8, 1))):
                        mtok = nr * wl1
                        for cj in range(4):
                            tp = pstp.tile([128, 128], F32, tag="tr")
                            nc.tensor.transpose(
                                tp[:, :mtok],
                                x2tok[mi][:mtok, cj * 128 : (cj + 1) * 128],
                                ident[:mtok, :mtok],
                            )
                            nc.any.tensor_copy(
                                nxt[cj][:, mi * 128 : mi * 128 + mtok], tp[:, :mtok]
                            )
                    xT = nxt
                else:
                    # final merge: token-major [64, 1024], straight to DRAM.
                    # wm2 (8 MB) is streamed; k outer so each chunk loads once.
                    nk = 4 * d // 128  # 16
                    fstg = constp.tile([T_MG[2], d2], F32, tag="fin", name="fstg")
                    ps0 = psp.tile([T_MG[2], 512], F32, tag="mm")
                    ps1 = psp.tile([T_MG[2], 512], F32, tag="yp")
                    for k in range(nk):
                        g, ci = divmod(k, nct)
                        wmk = wm2p.tile([128, d2], F32R, tag="wm2")
                        nc.gpsimd.dma_start(
                            out=wmk[:], in_=wm_d[2][k * 128 : (k + 1) * 128, :]
                        )
                        cg = blkp.tile([128, T_MG[2]], BF16, tag="mgl2", name="mgl2")
                        nc.vector.tensor_copy(
                            cg[:].rearrange("c (r w) -> c r w", r=4),
                            gview[g][ci][:],
                        )
                        nc.tensor.matmul(
                            ps0[:],
                            cg[:],
                            wmk[:, :512],
                            start=(k == 0),
                            stop=(k == nk - 1),
                        )
                        nc.tensor.matmul(
                            ps1[:],
                            cg[:],
                            wmk[:, 512:],
                            start=(k == 0),
                            stop=(k == nk - 1),
                        )
                    nc.vector.tensor_add(fstg[:, :512], ps0[:], bm2_s[:, :512])
                    nc.vector.tensor_add(fstg[:, 512:], ps1[:], bm2_s[:, 512:])
                    nc.sync.dma_start(out=fin_d[:], in_=fstg[:])

    nc.compile()
    return nc


_NC_CACHE = {}


def _get_nc():
    if "nc" not in _NC_CACHE:
        _NC_CACHE["nc"] = _build_program()
    return _NC_CACHE["nc"]


def _host_prep(params):
    """Per-level host-side weight folding (cheap, O(d^3) numpy)."""
    prep = []
    kk = np.arange(WD)[None, :] - np.arange(TB)[:, None]  # j - i
    band = (kk >= 0) & (kk < K)
    for l in range(NLEV):
        p = params[l]
        wq = np.asarray(p["wq"], np.float32)
        wk = np.asarray(p["wk"], np.float32)
        wv = np.asarray(p["wv"], np.float32)
        wo = np.asarray(p["wo"], np.float32)
        bq = np.asarray(p["bq"], np.float32)
        bv = np.asarray(p["bv"], np.float32)
        bo = np.asarray(p["bo"], np.float32)
        rb = np.asarray(p["rel_bias"], np.float32)[0]  # [K]
        wqk = (wq @ wk.T).astype(np.float32)
        bqk = (wk @ bq).astype(np.float32)
        wvo = (wv @ wo).astype(np.float32)
        bvo = (bv @ wo + bo).astype(np.float32)
        mexp = np.zeros((TB, WD), np.float32)
        mexp[band] = np.exp(rb / np.sqrt(np.float32(D[l])))[kk[band]]
        mexp = np.tile(mexp, (1, 4))
        wm = np.asarray(p["w_merge"], np.float32)
        # fold the attention output projection into the merge weight:
        # concat_g(Y_g Wvo + bvo) @ wm = concat_g(Y_g) @ wm_eff + bm
        d = D[l]
        wm_eff = np.vstack([wvo @ wm[g * d : (g + 1) * d] for g in range(4)]).astype(
            np.float32
        )
        bm = (np.tile(bvo, 4) @ wm).astype(np.float32)
        if l == 2:
            import ml_dtypes

            wm_eff = wm_eff.astype(ml_dtypes.bfloat16)
        prep.append((wqk, bqk, wvo, bvo, mexp, wm_eff, bm))
    return prep


def kernel(x, params):
    from concourse.bass_utils import run_bass_kernel_spmd

    x = np.asarray(x, np.float32)
    prep = _host_prep(params)
    nc = _get_nc()

    in_maps = []
    for core in range(8):
        b, s = divmod(core, 4)
        rows = np.arange(OWN_ROWS * s, OWN_ROWS * s + ROWS_IN[0]) % GRID
        x_sh = x[b].reshape(GRID, GRID, C0)[rows].reshape(T_IN[0], C0)
        m = {"x": np.ascontiguousarray(x_sh)}
        for l in range(NLEV):
            wqk, bqk, wvo, bvo, mexp, wm = prep[l]
            m[f"wqk{l}"] = wqk
            m[f"bqk{l}"] = bqk
            m[f"wvo{l}"] = wvo
            m[f"bvo{l}"] = bvo
            m[f"mexp{l}"] = mexp
            m[f"wm{l}"] = wm
        in_maps.append(m)

    res = run_bass_kernel_spmd(nc, in_maps, list(range(8)))

    skips = [
        np.empty((B, GRID * GRID // 4**l, D[l]), np.float32) for l in range(NLEV)
    ]
    fin = np.empty((B, 256, 1024), np.float32)
    for core in range(8):
        b, s = divmod(core, 4)
        r = res.results[core]
        for l in range(NLEV):
            ts = T_SKIP[l]
            skips[l][b, s * ts : (s + 1) * ts] = r[f"skip{l}"].T
        fin[b, s * 64 : (s + 1) * 64] = r["fin"]
    return (fin, skips[0], skips[1], skips[2])


# revision 35
# speedup vs baseline: 1.0011x; 1.0011x over previous
"""Trainium2 Bass kernel for nn_Encoder_66065186947370 (3-level neighbor-attention encoder).

Sharding: B(2) x H-rows(4) = 8 cores. Each core computes its 32 H-rows of the
output plus a forward halo (circular window of 8 => 7-token halo per level,
taken at full-grid-row granularity), so no inter-core communication is needed.

On-chip layout is feature-major ([channel, token]); neighbor attention is done
in 120-token blocks as dense PE matmuls against a 127-wide position window with
a multiplicative exp-mask (band structure + exp(rel_bias/sqrt(C)) folded in,
off-band exactly 0). Algebraic folds (host-side, weights only):
  Wqk = wq @ wk.T  (scores = (x Wqk + wk bq) . x  up to softmax-invariant terms)
  Wvo = wv @ wo,  bvo = bv @ wo + bo   (attn = (P.X) Wvo + bvo, since P rows sum to 1)
"""

import sys

sys.path.insert(0, "/opt/trn_rl_repo")

import numpy as np

import concourse.bass as bass
import concourse.tile as tile
from concourse import bacc, mybir
from concourse.masks import make_identity

F32 = mybir.dt.float32
F32R = mybir.dt.float32r
BF16 = mybir.dt.bfloat16


def _r(ap):
    """Bitcast an fp32 AP to float32r for single-pass PE streaming (fast for
    moving dim >= 256; products rounded TF32-style on HW)."""
    return ap.bitcast(F32R)
AF = mybir.ActivationFunctionType
ALU = mybir.AluOpType

# ---- static problem geometry -------------------------------------------------
B, GRID, C0 = 2, 128, 128
NLEV = 3
K = 8  # window
TB = 120  # attention block tokens
WD = TB + K - 1  # 127 position window

D = [128, 256, 512]  # channels per level
W = [128, 64, 32]  # grid width per level
ROWS_IN = [39, 19, 9]  # grid rows of level input held per core (own + halo)
ROWS_NA = [38, 18, 8]  # grid rows of NA output computed per core
T_IN = [ROWS_IN[l] * W[l] for l in range(3)]  # 4992, 1216, 288
T_NA = [ROWS_NA[l] * W[l] for l in range(3)]  # 4864, 1152, 256
T_SKIP = [4096, 1024, 256]  # own NA tokens stored as skip output
T_MG = [1216, 288, 64]  # merge output tokens per core
OWN_ROWS = 32  # own L0 grid rows per core


def _ceil_div(a, b):
    return (a + b - 1) // b


def _build_program():
    nc = bacc.Bacc(None, target_bir_lowering=False)

    # ---- DRAM I/O ----
    x_d = nc.dram_tensor("x", [T_IN[0], C0], F32, kind="ExternalInput")
    wqk_d, wvo_d, bqk_d, bvo_d, mex_d, wm_d = [], [], [], [], [], []
    for l in range(NLEV):
        d = D[l]
        wqk_d.append(nc.dram_tensor(f"wqk{l}", [d, d], F32R, kind="ExternalInput"))
        wvo_d.append(nc.dram_tensor(f"wvo{l}", [d, d], F32R, kind="ExternalInput"))
        bqk_d.append(nc.dram_tensor(f"bqk{l}", [d], F32, kind="ExternalInput"))
        bvo_d.append(nc.dram_tensor(f"bvo{l}", [d], F32, kind="ExternalInput"))
        mex_d.append(nc.dram_tensor(f"mexp{l}", [TB, 4 * WD], F32, kind="ExternalInput"))
        wm_d.append(
            nc.dram_tensor(
                f"wm{l}", [4 * d, 2 * d], BF16 if l == 2 else F32R,
                kind="ExternalInput",
            )
        )
    skip_d = [
        nc.dram_tensor(f"skip{l}", [D[l], T_SKIP[l]], F32, kind="ExternalOutput")
        for l in range(NLEV)
    ]
    fin_d = nc.dram_tensor("fin", [T_MG[2], 2 * D[2]], F32, kind="ExternalOutput")

    with tile.TileContext(nc) as tc:
        with (
            tc.tile_pool(name="const", bufs=1) as constp,
            tc.tile_pool(name="wts", bufs=1) as wtsp,
            tc.tile_pool(name="big", bufs=1) as bigp,
            tc.tile_pool(name="wm2s", bufs=14) as wm2p,
            tc.tile_pool(name="blk", bufs=3) as blkp,
            tc.tile_pool(name="ps", bufs=2, space="PSUM") as psp,
            tc.tile_pool(name="pst", bufs=2, space="PSUM") as pstp,
        ):
            ident = constp.tile([128, 128], F32)
            make_identity(nc, ident[:])
            identb = constp.tile([128, 128], BF16)
            make_identity(nc, identb[:])

            # ---- load weights/constants (wm2 streamed later) ----
            wqk_s, wvo_s, bqk_s, bvo_s, mex_s, wm_s = [], [], [], [], [], []
            for l in range(NLEV):
                d = D[l]
                nct = d // 128
                wq = wtsp.tile([128, nct * d], F32R, tag=f"wqk{l}")
                wv = wtsp.tile([128, nct * d], F32R, tag=f"wvo{l}")
                nc.gpsimd.dma_start(
                    out=wq[:].rearrange("p (k dd) -> p k dd", k=nct),
                    in_=wqk_d[l][:].rearrange("(k p) dd -> p k dd", p=128),
                )
                nc.gpsimd.dma_start(
                    out=wv[:].rearrange("p (k dd) -> p k dd", k=nct),
                    in_=wvo_d[l][:].rearrange("(k p) dd -> p k dd", p=128),
                )
                bq = wtsp.tile([128, nct], F32, tag=f"bqk{l}")
                bv = wtsp.tile([128, nct], F32, tag=f"bvo{l}")
                nc.gpsimd.dma_start(
                    out=bq[:], in_=bqk_d[l][:].rearrange("(k p) -> p k", p=128)
                )
                nc.gpsimd.dma_start(
                    out=bv[:], in_=bvo_d[l][:].rearrange("(k p) -> p k", p=128)
                )
                me = wtsp.tile([TB, 4 * WD], F32, tag=f"mexp{l}")
                nc.gpsimd.dma_start(out=me[:], in_=mex_d[l][:])
                if l < 2:
                    d2 = 2 * d
                    nk = 4 * d // 128
                    wm = wtsp.tile([128, nk * d2], F32R, tag=f"wm{l}")
                    nc.gpsimd.dma_start(
                        out=wm[:].rearrange("p (k d2) -> p k d2", k=nk),
                        in_=wm_d[l][:].rearrange("(k p) d2 -> p k d2", p=128),
                    )
                else:
                    wm = None
                wqk_s.append(wq)
                wvo_s.append(wv)
                bqk_s.append(bq)
                bvo_s.append(bv)
                mex_s.append(me)
                wm_s.append(wm)

            # ---- level-0 input: load token-major, PE-transpose to feature-major
            xT = [bigp.tile([128, T_IN[0]], F32, tag="xT_0")]
            for t in range(ROWS_IN[0]):
                xt = blkp.tile([128, 128], F32, tag="xin")
                nc.sync.dma_start(out=xt[:], in_=x_d[t * 128 : (t + 1) * 128, :])
                tp = pstp.tile([128, 128], F32, tag="tr")
                nc.tensor.transpose(tp[:], xt[:], ident[:])
                nc.any.tensor_copy(xT[0][:, t * 128 : (t + 1) * 128], tp[:])

            for l in range(NLEV):
                d = D[l]
                nct = d // 128
                t_in, t_na = T_IN[l], T_NA[l]
                inv_sqrt = 1.0 / float(np.sqrt(d))

                # ---- phase A: Q'^T = Wqk^T . x^T (+bias), feature-major
                qT = [
                    bigp.tile([128, t_na], F32, tag=f"big1_{ci}") for ci in range(nct)
                ]
                for cj in range(nct):
                    for t0 in range(0, t_na, 512):
                        n = min(512, t_na - t0)
                        ps = psp.tile([128, 512], F32, tag="mm")
                        for ci in range(nct):
                            nc.tensor.matmul(
                                ps[:, :n],
                                wqk_s[l][:, ci * d + cj * 128 : ci * d + cj * 128 + 128],
                                xT[ci][:, t0 : t0 + n],
                                start=(ci == 0),
                                stop=(ci == nct - 1),
                            )
                        nc.scalar.activation(
                            qT[cj][:, t0 : t0 + n],
                            ps[:, :n],
                            AF.Identity,
                            bias=bqk_s[l][:, cj : cj + 1],
                        )

                # ---- phase B: neighbor attention blocks -> Y^T feature-major
                yT = [
                    bigp.tile([128, t_na], F32, tag=f"big2_{ci}") for ci in range(nct)
                ]
                # attention runs in groups of up to 4 full blocks so the
                # softmax-side DVE/ACT ops work on wide tiles (one PSUM bank
                # holds 4 blocks of scores); the ragged tail block runs alone.
                nfull = t_na // TB
                groups = [
                    (g0, min(4, nfull - g0)) for g0 in range(0, nfull, 4)
                ]
                if t_na % TB:
                    groups.append((nfull, 0))  # tail marker
                for g0, gn in groups:
                    if gn == 0:
                        gn, m, w = 1, t_na - g0 * TB, min(WD, t_in - g0 * TB)
                    else:
                        m, w = TB, WD
                    gw, gm = gn * WD, gn * TB
                    sc = psp.tile([TB, 4 * WD], F32, tag="mm")
                    for j in range(gn):
                        tb = (g0 + j) * TB
                        for ci in range(nct):
                            nc.tensor.matmul(
                                sc[:m, j * WD : j * WD + w],
                                qT[ci][:, tb : tb + m],
                                xT[ci][:, tb : tb + w].bitcast(F32),
                                start=(ci == 0),
                                stop=(ci == nct - 1),
                            )
                    # column range actually populated (tail: w < WD)
                    cw = (gn - 1) * WD + w
                    ex = blkp.tile([TB, 4 * WD], F32, tag="ex")
                    nc.scalar.activation(
                        ex[:m, :cw], sc[:m, :cw], AF.Exp, scale=inv_sqrt
                    )
                    en = blkp.tile([TB, 4 * WD], F32, tag="en")
                    # NB: fused tensor_tensor_reduce(accum_out=...) wedges the
                    # device on this walrus version; use two DVE ops instead.
                    nc.vector.tensor_mul(en[:m, :cw], ex[:m, :cw], mex_s[l][:m, :cw])
                    rs = blkp.tile([TB, 4], F32, tag="rs")
                    nc.vector.tensor_reduce(
                        rs[:m, :gn],
                        en[:m, :cw].rearrange("p (g w) -> p g w", g=gn)
                        if w == WD
                        else en[:m, :cw].unsqueeze(1),
                        axis=mybir.AxisListType.X,
                        op=ALU.add,
                    )
                    ri = blkp.tile([TB, 4], F32, tag="ri")
                    nc.vector.reciprocal(ri[:m, :gn], rs[:m, :gn])
                    ep = blkp.tile([TB, 4 * WD], BF16, tag="ep")
                    nc.vector.tensor_tensor(
                        ep[:m, :cw].rearrange("p (g w) -> p g w", g=gn)
                        if w == WD
                        else ep[:m, :cw].unsqueeze(1),
                        en[:m, :cw].rearrange("p (g w) -> p g w", g=gn)
                        if w == WD
                        else en[:m, :cw].unsqueeze(1),
                        ri[:m, :gn].unsqueeze(2).broadcast_to((m, gn, w)),
                        ALU.mult,
                    )
                    # transpose normalized probs per block: [m, w] -> [w, m]
                    et_ps = pstp.tile([WD, 4 * TB], BF16, tag="trb", name="et_ps")
                    for j in range(gn):
                        nc.tensor.transpose(
                            et_ps[:w, j * TB : j * TB + m],
                            ep[:m, j * WD : j * WD + w],
                            identb[:m, :m],
                        )
                    et = blkp.tile([WD, 4 * TB], BF16, tag="et")
                    nc.any.tensor_copy(et[:w, : (gn - 1) * TB + m], et_ps[:w, : (gn - 1) * TB + m])
                    for ci in range(nct):
                        xw_ps = pstp.tile([WD, 4 * 128], F32, tag="tr")
                        for j in range(gn):
                            tb = (g0 + j) * TB
                            nc.tensor.transpose(
                                xw_ps[:w, j * 128 : j * 128 + 128],
                                xT[ci][:, tb : tb + w].bitcast(F32),
                                ident[:],
                            )
                        xw = blkp.tile([WD, 4 * 128], BF16, tag="xw")
                        nc.any.tensor_copy(
                            xw[:w, : gn * 128], xw_ps[:w, : gn * 128]
                        )
                        yp = psp.tile([128, 4 * TB], F32, tag="yp")
                        for j in range(gn):
                            nc.tensor.matmul(
                                yp[:, j * TB : j * TB + m],
                                xw[:w, j * 128 : j * 128 + 128],
                                et[:w, j * TB : j * TB + m],
                                start=True,
                                stop=True,
                            )
                        nc.any.tensor_copy(
                            yT[ci][:, g0 * TB : g0 * TB + (gn - 1) * TB + m],
                            yp[:, : (gn - 1) * TB + m],
                        )

                # ---- phase C: attn^T = Wvo^T . Y^T (+bvo), feature-major
                aT = [
                    bigp.tile([128, t_na], F32, tag=f"big1_{ci}") for ci in range(nct)
                ]
                for cj in range(nct):
                    for t0 in range(0, t_na, 512):
                        n = min(512, t_na - t0)
                        ps = psp.tile([128, 512], F32, tag="mm")
                        for ci in range(nct):
                            nc.tensor.matmul(
                                ps[:, :n],
                                wvo_s[l][:, ci * d + cj * 128 : ci * d + cj * 128 + 128],
                                yT[ci][:, t0 : t0 + n],
                                start=(ci == 0),
                                stop=(ci == nct - 1),
                            )
                        if (t0 // 512) % 2 == 1:
                            nc.scalar.activation(
                                aT[cj][:, t0 : t0 + n],
                                ps[:, :n],
                                AF.Identity,
                                bias=bvo_s[l][:, cj : cj + 1],
                            )
                        else:
                            nc.vector.tensor_scalar_add(
                                aT[cj][:, t0 : t0 + n],
                                ps[:, :n],
                                bvo_s[l][:, cj : cj + 1],
                            )

                # ---- phase D: skip output, stored feature-major (host transposes)
                for cj in range(nct):
                    nc.sync.dma_start(
                        out=skip_d[l][cj * 128 : (cj + 1) * 128, :],
                        in_=aT[cj][:, : T_SKIP[l]].bitcast(F32),
                    )

                # ---- phase E: patch merge
                d2 = 2 * d
                rows_na = ROWS_NA[l]
                wl1 = W[l] // 2  # next-level grid width
                # group g reads (2h+a, 2w+b); channel row in w_merge is g*d + c
                gview = [
                    [
                        aT[ci].rearrange("c (h w) -> c h w", h=rows_na)[:, a::2, b::2]
                        for ci in range(nct)
                    ]
                    for (a, b) in ((0, 0), (1, 0), (0, 1), (1, 1))
                ]
                if l == 0:
                    # feature-major output x1T [2][128, T_MG[0]]
                    nxt = [
                        bigp.tile([128, T_MG[0]], F32, tag=f"xT_{cj}")
                        for cj in range(2)
                    ]
                    nrows = T_MG[0] // wl1  # 19
                    for cj in range(2):
                        for r0 in range(0, nrows, 8):
                            nr = min(8, nrows - r0)
                            n = nr * wl1
                            ps = psp.tile([128, 512], F32, tag="mm")
                            for g in range(4):
                                nc.tensor.matmul(
                                    ps[:, :n],
                                    wm_s[0][
                                        :, g * d2 + cj * 128 : g * d2 + cj * 128 + 128
                                    ],
                                    gview[g][0][:, r0 : r0 + nr, :],
                                    start=(g == 0),
                                    stop=(g == 3),
                                )
                            nc.any.tensor_copy(
                                nxt[cj][:, r0 * wl1 : r0 * wl1 + n], ps[:, :n]
                            )
                    xT = nxt
                elif l == 1:
                    # token-major output x2tok [3][<=128, 512]
                    x2tok = [
                        bigp.tile([128, d2], F32, tag=f"big2_{mi}") for mi in range(3)
                    ]
                    nk = 4 * d // 128  # 8
                    for mi, (r0, nr) in enumerate(((0, 4), (4, 4), (8, 1))):
                        mtok = nr * wl1
                        ps = psp.tile([128, 512], F32, tag="mm")
                        for k in range(nk):
                            g, ci = divmod(k, nct)
                            # stationary operand must have a single free dim:
                            # stage the strided view through a contiguous tile
                            cg = blkp.tile([128, 128], F32R, tag="mgl", name="mgl")
                            nc.vector.tensor_copy(
                                cg[:, :mtok].rearrange("c (r w) -> c r w", r=nr),
                                gview[g][ci][:, r0 : r0 + nr, :],
                            )
                            nc.tensor.matmul(
                                ps[:mtok, :],
                                cg[:, :mtok],
                                wm_s[1][:, k * d2 : (k + 1) * d2],
                                start=(k == 0),
                                stop=(k == nk - 1),
                            )
                        nc.any.tensor_copy(x2tok[mi][:mtok, :], ps[:mtok, :])
                    # transpose to feature-major xT2 [4][128, 288]
                    nxt = [
                        bigp.tile([128, T_IN[2]], F32, tag=f"xT_{cj}")
                        for cj in range(4)
                    ]
                    for mi, (r0, nr) in enumerate(((0, 4), (4, 4), (8, 1))):
                        mtok = nr * wl1
                        for cj in range(4):
                            tp = pstp.tile([128, 128], F32, tag="tr")
                            nc.tensor.transpose(
                                tp[:, :mtok],
                                x2tok[mi][:mtok, cj * 128 : (cj + 1) * 128],
                                ident[:mtok, :mtok],
                            )
                            nc.any.tensor_copy(
                                nxt[cj][:, mi * 128 : mi * 128 + mtok], tp[:, :mtok]
                            )
                    xT = nxt
                else:
                    # final merge: token-major [64, 1024], straight to DRAM.
                    # wm2 (8 MB) is streamed; k outer so each chunk loads once.
                    nk = 4 * d // 128  # 16
                    fstg = constp.tile([T_MG[2], d2], F32, tag="fin", name="fstg")
                    ps0 = psp.tile([T_MG[2], 512], F32, tag="mm")
                    ps1 = psp.tile([T_MG[2], 512], F32, tag="yp")
                    for k in range(nk):
                        g, ci = divmod(k, nct)
                        wmk = wm2p.tile([128, d2], F32R, tag="wm2")
                        nc.gpsimd.dma_start(
                            out=wmk[:], in_=wm_d[2][k * 128 : (k + 1) * 128, :]
                        )
                        cg = blkp.tile([128, T_MG[2]], BF16, tag="mgl2", name="mgl2")
                        nc.vector.tensor_copy(
                            cg[:].rearrange("c (r w) -> c r w", r=4),
                            gview[g][ci][:],
                        )
                        nc.tensor.matmul(
                            ps0[:],
                            cg[:],
                            wmk[:, :512],
                            start=(k == 0),
                            stop=(k == nk - 1),
                        )
                        nc.tensor.matmul(
                            ps1[:],
                            cg[:],
                            wmk[:, 512:],
                            start=(k == 0),
                            stop=(k == nk - 1),
                        )
                    nc.any.tensor_copy(fstg[:, :512], ps0[:])
                    nc.any.tensor_copy(fstg[:, 512:], ps1[:])
                    nc.sync.dma_start(out=fin_d[:], in_=fstg[:])

    nc.compile()
    return nc


_NC_CACHE = {}


def _get_nc():
    if "nc" not in _NC_CACHE:
        _NC_CACHE["nc"] = _build_program()
    return _NC_CACHE["nc"]


def _host_prep(params):
    """Per-level host-side weight folding (cheap, O(d^3) numpy)."""
    prep = []
    kk = np.arange(WD)[None, :] - np.arange(TB)[:, None]  # j - i
    band = (kk >= 0) & (kk < K)
    for l in range(NLEV):
        p = params[l]
        wq = np.asarray(p["wq"], np.float32)
        wk = np.asarray(p["wk"], np.float32)
        wv = np.asarray(p["wv"], np.float32)
        wo = np.asarray(p["wo"], np.float32)
        bq = np.asarray(p["bq"], np.float32)
        bv = np.asarray(p["bv"], np.float32)
        bo = np.asarray(p["bo"], np.float32)
        rb = np.asarray(p["rel_bias"], np.float32)[0]  # [K]
        wqk = (wq @ wk.T).astype(np.float32)
        bqk = (wk @ bq).astype(np.float32)
        wvo = (wv @ wo).astype(np.float32)
        bvo = (bv @ wo + bo).astype(np.float32)
        mexp = np.zeros((TB, WD), np.float32)
        mexp[band] = np.exp(rb / np.sqrt(np.float32(D[l])))[kk[band]]
        mexp = np.tile(mexp, (1, 4))
        wm = np.asarray(p["w_merge"], np.float32)
        if l == 2:
            import ml_dtypes

            wm = wm.astype(ml_dtypes.bfloat16)
        prep.append((wqk, bqk, wvo, bvo, mexp, wm))
    return prep


def kernel(x, params):
    from concourse.bass_utils import run_bass_kernel_spmd

    x = np.asarray(x, np.float32)
    prep = _host_prep(params)
    nc = _get_nc()

    in_maps = []
    for core in range(8):
        b, s = divmod(core, 4)
        rows = np.arange(OWN_ROWS * s, OWN_ROWS * s + ROWS_IN[0]) % GRID
        x_sh = x[b].reshape(GRID, GRID, C0)[rows].reshape(T_IN[0], C0)
        m = {"x": np.ascontiguousarray(x_sh)}
        for l in range(NLEV):
            wqk, bqk, wvo, bvo, mexp, wm = prep[l]
            m[f"wqk{l}"] = wqk
            m[f"bqk{l}"] = bqk
            m[f"wvo{l}"] = wvo
            m[f"bvo{l}"] = bvo
            m[f"mexp{l}"] = mexp
            m[f"wm{l}"] = wm
        in_maps.append(m)

    res = run_bass_kernel_spmd(nc, in_maps, list(range(8)))

    skips = [
        np.empty((B, GRID * GRID // 4**l, D[l]), np.float32) for l in range(NLEV)
    ]
    fin = np.empty((B, 256, 1024), np.float32)
    for core in range(8):
        b, s = divmod(core, 4)
        r = res.results[core]
        for l in range(NLEV):
            ts = T_SKIP[l]
            skips[l][b, s * ts : (s + 1) * ts] = r[f"skip{l}"].T
        fin[b, s * 64 : (s + 1) * 64] = r["fin"]
    return (fin, skips[0], skips[1], skips[2])


# revision 36
# speedup vs baseline: 1.0060x; 1.0049x over previous
"""Trainium2 Bass kernel for nn_Encoder_66065186947370 (3-level neighbor-attention encoder).

Sharding: B(2) x H-rows(4) = 8 cores. Each core computes its 32 H-rows of the
output plus a forward halo (circular window of 8 => 7-token halo per level,
taken at full-grid-row granularity), so no inter-core communication is needed.

On-chip layout is feature-major ([channel, token]); neighbor attention is done
in 120-token blocks as dense PE matmuls against a 127-wide position window with
a multiplicative exp-mask (band structure + exp(rel_bias/sqrt(C)) folded in,
off-band exactly 0). Algebraic folds (host-side, weights only):
  Wqk = wq @ wk.T  (scores = (x Wqk + wk bq) . x  up to softmax-invariant terms)
  Wvo = wv @ wo,  bvo = bv @ wo + bo   (attn = (P.X) Wvo + bvo, since P rows sum to 1)
"""

import sys

sys.path.insert(0, "/opt/trn_rl_repo")

import numpy as np

import concourse.bass as bass
import concourse.tile as tile
from concourse import bacc, mybir
from concourse.masks import make_identity

F32 = mybir.dt.float32
F32R = mybir.dt.float32r
BF16 = mybir.dt.bfloat16


def _r(ap):
    """Bitcast an fp32 AP to float32r for single-pass PE streaming (fast for
    moving dim >= 256; products rounded TF32-style on HW)."""
    return ap.bitcast(F32R)
AF = mybir.ActivationFunctionType
ALU = mybir.AluOpType

# ---- static problem geometry -------------------------------------------------
B, GRID, C0 = 2, 128, 128
NLEV = 3
K = 8  # window
TB = 120  # attention block tokens
WD = TB + K - 1  # 127 position window

D = [128, 256, 512]  # channels per level
W = [128, 64, 32]  # grid width per level
ROWS_IN = [39, 19, 9]  # grid rows of level input held per core (own + halo)
ROWS_NA = [38, 18, 8]  # grid rows of NA output computed per core
T_IN = [ROWS_IN[l] * W[l] for l in range(3)]  # 4992, 1216, 288
T_NA = [ROWS_NA[l] * W[l] for l in range(3)]  # 4864, 1152, 256
T_SKIP = [4096, 1024, 256]  # own NA tokens stored as skip output
T_MG = [1216, 288, 64]  # merge output tokens per core
OWN_ROWS = 32  # own L0 grid rows per core


def _ceil_div(a, b):
    return (a + b - 1) // b


def _build_program():
    nc = bacc.Bacc(None, target_bir_lowering=False)

    # ---- DRAM I/O ----
    x_d = nc.dram_tensor("x", [T_IN[0], C0], F32R, kind="ExternalInput")
    wqk_d, wvo_d, bqk_d, bvo_d, mex_d, wm_d = [], [], [], [], [], []
    for l in range(NLEV):
        d = D[l]
        wqk_d.append(nc.dram_tensor(f"wqk{l}", [d, d], F32R, kind="ExternalInput"))
        wvo_d.append(nc.dram_tensor(f"wvo{l}", [d, d], F32R, kind="ExternalInput"))
        bqk_d.append(nc.dram_tensor(f"bqk{l}", [d], F32, kind="ExternalInput"))
        bvo_d.append(nc.dram_tensor(f"bvo{l}", [d], F32, kind="ExternalInput"))
        mex_d.append(nc.dram_tensor(f"mexp{l}", [TB, 4 * WD], F32, kind="ExternalInput"))
        wm_d.append(
            nc.dram_tensor(
                f"wm{l}", [4 * d, 2 * d], BF16 if l == 2 else F32R,
                kind="ExternalInput",
            )
        )
    skip_d = [
        nc.dram_tensor(f"skip{l}", [D[l], T_SKIP[l]], F32, kind="ExternalOutput")
        for l in range(NLEV)
    ]
    fin_d = nc.dram_tensor("fin", [T_MG[2], 2 * D[2]], F32, kind="ExternalOutput")

    with tile.TileContext(nc) as tc:
        with (
            tc.tile_pool(name="const", bufs=1) as constp,
            tc.tile_pool(name="wts", bufs=1) as wtsp,
            tc.tile_pool(name="big", bufs=1) as bigp,
            tc.tile_pool(name="wm2s", bufs=14) as wm2p,
            tc.tile_pool(name="blk", bufs=3) as blkp,
            tc.tile_pool(name="ps", bufs=2, space="PSUM") as psp,
            tc.tile_pool(name="pst", bufs=2, space="PSUM") as pstp,
        ):
            ident = constp.tile([128, 128], F32)
            make_identity(nc, ident[:])
            identb = constp.tile([128, 128], BF16)
            make_identity(nc, identb[:])
            identr = constp.tile([128, 128], F32R)
            make_identity(nc, identr[:])

            # ---- load weights/constants (wm2 streamed later) ----
            wqk_s, wvo_s, bqk_s, bvo_s, mex_s, wm_s = [], [], [], [], [], []
            for l in range(NLEV):
                d = D[l]
                nct = d // 128
                wq = wtsp.tile([128, nct * d], F32R, tag=f"wqk{l}")
                wv = wtsp.tile([128, nct * d], F32R, tag=f"wvo{l}")
                nc.gpsimd.dma_start(
                    out=wq[:].rearrange("p (k dd) -> p k dd", k=nct),
                    in_=wqk_d[l][:].rearrange("(k p) dd -> p k dd", p=128),
                )
                nc.gpsimd.dma_start(
                    out=wv[:].rearrange("p (k dd) -> p k dd", k=nct),
                    in_=wvo_d[l][:].rearrange("(k p) dd -> p k dd", p=128),
                )
                bq = wtsp.tile([128, nct], F32, tag=f"bqk{l}")
                bv = wtsp.tile([128, nct], F32, tag=f"bvo{l}")
                nc.gpsimd.dma_start(
                    out=bq[:], in_=bqk_d[l][:].rearrange("(k p) -> p k", p=128)
                )
                nc.gpsimd.dma_start(
                    out=bv[:], in_=bvo_d[l][:].rearrange("(k p) -> p k", p=128)
                )
                me = wtsp.tile([TB, 4 * WD], F32, tag=f"mexp{l}")
                nc.gpsimd.dma_start(out=me[:], in_=mex_d[l][:])
                if l < 2:
                    d2 = 2 * d
                    nk = 4 * d // 128
                    wm = wtsp.tile([128, nk * d2], F32R, tag=f"wm{l}")
                    nc.gpsimd.dma_start(
                        out=wm[:].rearrange("p (k d2) -> p k d2", k=nk),
                        in_=wm_d[l][:].rearrange("(k p) d2 -> p k d2", p=128),
                    )
                else:
                    wm = None
                wqk_s.append(wq)
                wvo_s.append(wv)
                bqk_s.append(bq)
                bvo_s.append(bv)
                mex_s.append(me)
                wm_s.append(wm)

            # ---- level-0 input: load token-major, PE-transpose to feature-major
            xT = [bigp.tile([128, T_IN[0]], F32, tag="xT_0")]
            for t in range(ROWS_IN[0]):
                xt = blkp.tile([128, 128], F32, tag="xin")
                nc.sync.dma_start(out=xt[:], in_=x_d[t * 128 : (t + 1) * 128, :])
                tp = pstp.tile([128, 128], F32, tag="tr")
                nc.tensor.transpose(tp[:], xt[:], ident[:])
                nc.any.tensor_copy(xT[0][:, t * 128 : (t + 1) * 128], tp[:])

            for l in range(NLEV):
                d = D[l]
                nct = d // 128
                t_in, t_na = T_IN[l], T_NA[l]
                inv_sqrt = 1.0 / float(np.sqrt(d))

                # ---- phase A: Q'^T = Wqk^T . x^T (+bias), feature-major
                qT = [
                    bigp.tile([128, t_na], F32, tag=f"big1_{ci}") for ci in range(nct)
                ]
                for cj in range(nct):
                    for t0 in range(0, t_na, 512):
                        n = min(512, t_na - t0)
                        ps = psp.tile([128, 512], F32, tag="mm")
                        for ci in range(nct):
                            nc.tensor.matmul(
                                ps[:, :n],
                                wqk_s[l][:, ci * d + cj * 128 : ci * d + cj * 128 + 128],
                                xT[ci][:, t0 : t0 + n],
                                start=(ci == 0),
                                stop=(ci == nct - 1),
                            )
                        nc.scalar.activation(
                            qT[cj][:, t0 : t0 + n],
                            ps[:, :n],
                            AF.Identity,
                            bias=bqk_s[l][:, cj : cj + 1],
                        )

                # ---- phase B: neighbor attention blocks -> Y^T feature-major
                yT = [
                    bigp.tile([128, t_na], F32, tag=f"big2_{ci}") for ci in range(nct)
                ]
                # attention runs in groups of up to 4 full blocks so the
                # softmax-side DVE/ACT ops work on wide tiles (one PSUM bank
                # holds 4 blocks of scores); the ragged tail block runs alone.
                nfull = t_na // TB
                groups = [
                    (g0, min(4, nfull - g0)) for g0 in range(0, nfull, 4)
                ]
                if t_na % TB:
                    groups.append((nfull, 0))  # tail marker
                for g0, gn in groups:
                    if gn == 0:
                        gn, m, w = 1, t_na - g0 * TB, min(WD, t_in - g0 * TB)
                    else:
                        m, w = TB, WD
                    gw, gm = gn * WD, gn * TB
                    sc = psp.tile([TB, 4 * WD], F32, tag="mm")
                    for j in range(gn):
                        tb = (g0 + j) * TB
                        for ci in range(nct):
                            nc.tensor.matmul(
                                sc[:m, j * WD : j * WD + w],
                                qT[ci][:, tb : tb + m],
                                xT[ci][:, tb : tb + w].bitcast(F32),
                                start=(ci == 0),
                                stop=(ci == nct - 1),
                            )
                    # column range actually populated (tail: w < WD)
                    cw = (gn - 1) * WD + w
                    ex = blkp.tile([TB, 4 * WD], F32, tag="ex")
                    nc.scalar.activation(
                        ex[:m, :cw], sc[:m, :cw], AF.Exp, scale=inv_sqrt
                    )
                    en = blkp.tile([TB, 4 * WD], F32, tag="en")
                    # NB: fused tensor_tensor_reduce(accum_out=...) wedges the
                    # device on this walrus version; use two DVE ops instead.
                    nc.vector.tensor_mul(en[:m, :cw], ex[:m, :cw], mex_s[l][:m, :cw])
                    rs = blkp.tile([TB, 4], F32, tag="rs")
                    nc.vector.tensor_reduce(
                        rs[:m, :gn],
                        en[:m, :cw].rearrange("p (g w) -> p g w", g=gn)
                        if w == WD
                        else en[:m, :cw].unsqueeze(1),
                        axis=mybir.AxisListType.X,
                        op=ALU.add,
                    )
                    ri = blkp.tile([TB, 4], F32, tag="ri")
                    nc.vector.reciprocal(ri[:m, :gn], rs[:m, :gn])
                    ep = blkp.tile([TB, 4 * WD], BF16, tag="ep")
                    nc.vector.tensor_tensor(
                        ep[:m, :cw].rearrange("p (g w) -> p g w", g=gn)
                        if w == WD
                        else ep[:m, :cw].unsqueeze(1),
                        en[:m, :cw].rearrange("p (g w) -> p g w", g=gn)
                        if w == WD
                        else en[:m, :cw].unsqueeze(1),
                        ri[:m, :gn].unsqueeze(2).broadcast_to((m, gn, w)),
                        ALU.mult,
                    )
                    # transpose normalized probs per block: [m, w] -> [w, m]
                    et_ps = pstp.tile([WD, 4 * TB], BF16, tag="trb", name="et_ps")
                    for j in range(gn):
                        nc.tensor.transpose(
                            et_ps[:w, j * TB : j * TB + m],
                            ep[:m, j * WD : j * WD + w],
                            identb[:m, :m],
                        )
                    et = blkp.tile([WD, 4 * TB], BF16, tag="et")
                    nc.any.tensor_copy(et[:w, : (gn - 1) * TB + m], et_ps[:w, : (gn - 1) * TB + m])
                    for ci in range(nct):
                        xw_ps = pstp.tile([WD, 4 * 128], F32R, tag="tr")
                        for j in range(gn):
                            tb = (g0 + j) * TB
                            nc.tensor.transpose(
                                xw_ps[:w, j * 128 : j * 128 + 128],
                                xT[ci][:, tb : tb + w],
                                identr[:],
                            )
                        xw = blkp.tile([WD, 4 * 128], BF16, tag="xw")
                        nc.any.tensor_copy(
                            xw[:w, : gn * 128], xw_ps[:w, : gn * 128]
                        )
                        yp = psp.tile([128, 4 * TB], F32, tag="yp")
                        for j in range(gn):
                            nc.tensor.matmul(
                                yp[:, j * TB : j * TB + m],
                                xw[:w, j * 128 : j * 128 + 128],
                                et[:w, j * TB : j * TB + m],
                                start=True,
                                stop=True,
                            )
                        nc.any.tensor_copy(
                            yT[ci][:, g0 * TB : g0 * TB + (gn - 1) * TB + m],
                            yp[:, : (gn - 1) * TB + m],
                        )

                # ---- phase C: attn^T = Wvo^T . Y^T (+bvo), feature-major
                aT = [
                    bigp.tile([128, t_na], F32, tag=f"big1_{ci}") for ci in range(nct)
                ]
                for cj in range(nct):
                    for t0 in range(0, t_na, 512):
                        n = min(512, t_na - t0)
                        ps = psp.tile([128, 512], F32, tag="mm")
                        for ci in range(nct):
                            nc.tensor.matmul(
                                ps[:, :n],
                                wvo_s[l][:, ci * d + cj * 128 : ci * d + cj * 128 + 128],
                                yT[ci][:, t0 : t0 + n],
                                start=(ci == 0),
                                stop=(ci == nct - 1),
                            )
                        if (t0 // 512) % 2 == 1:
                            nc.scalar.activation(
                                aT[cj][:, t0 : t0 + n],
                                ps[:, :n],
                                AF.Identity,
                                bias=bvo_s[l][:, cj : cj + 1],
                            )
                        else:
                            nc.vector.tensor_scalar_add(
                                aT[cj][:, t0 : t0 + n],
                                ps[:, :n],
                                bvo_s[l][:, cj : cj + 1],
                            )

                # ---- phase D: skip output, stored feature-major (host transposes)
                for cj in range(nct):
                    nc.sync.dma_start(
                        out=skip_d[l][cj * 128 : (cj + 1) * 128, :],
                        in_=aT[cj][:, : T_SKIP[l]].bitcast(F32),
                    )

                # ---- phase E: patch merge
                d2 = 2 * d
                rows_na = ROWS_NA[l]
                wl1 = W[l] // 2  # next-level grid width
                # group g reads (2h+a, 2w+b); channel row in w_merge is g*d + c
                gview = [
                    [
                        aT[ci].rearrange("c (h w) -> c h w", h=rows_na)[:, a::2, b::2]
                        for ci in range(nct)
                    ]
                    for (a, b) in ((0, 0), (1, 0), (0, 1), (1, 1))
                ]
                if l == 0:
                    # feature-major output x1T [2][128, T_MG[0]]
                    nxt = [
                        bigp.tile([128, T_MG[0]], F32, tag=f"xT_{cj}")
                        for cj in range(2)
                    ]
                    nrows = T_MG[0] // wl1  # 19
                    for cj in range(2):
                        for r0 in range(0, nrows, 8):
                            nr = min(8, nrows - r0)
                            n = nr * wl1
                            ps = psp.tile([128, 512], F32, tag="mm")
                            for g in range(4):
                                nc.tensor.matmul(
                                    ps[:, :n],
                                    wm_s[0][
                                        :, g * d2 + cj * 128 : g * d2 + cj * 128 + 128
                                    ],
                                    gview[g][0][:, r0 : r0 + nr, :],
                                    start=(g == 0),
                                    stop=(g == 3),
                                )
                            nc.any.tensor_copy(
                                nxt[cj][:, r0 * wl1 : r0 * wl1 + n], ps[:, :n]
                            )
                    xT = nxt
                elif l == 1:
                    # token-major output x2tok [3][<=128, 512]
                    x2tok = [
                        bigp.tile([128, d2], F32, tag=f"big2_{mi}") for mi in range(3)
                    ]
                    nk = 4 * d // 128  # 8
                    for mi, (r0, nr) in enumerate(((0, 4), (4, 4), (8, 1))):
                        mtok = nr * wl1
                        ps = psp.tile([128, 512], F32, tag="mm")
                        for k in range(nk):
                            g, ci = divmod(k, nct)
                            # stationary operand must have a single free dim:
                            # stage the strided view through a contiguous tile
                            cg = blkp.tile([128, 128], F32R, tag="mgl", name="mgl")
                            nc.vector.tensor_copy(
                                cg[:, :mtok].rearrange("c (r w) -> c r w", r=nr),
                                gview[g][ci][:, r0 : r0 + nr, :],
                            )
                            nc.tensor.matmul(
                                ps[:mtok, :],
                                cg[:, :mtok],
                                wm_s[1][:, k * d2 : (k + 1) * d2],
                                start=(k == 0),
                                stop=(k == nk - 1),
                            )
                        nc.any.tensor_copy(x2tok[mi][:mtok, :], ps[:mtok, :])
                    # transpose to feature-major xT2 [4][128, 288]
                    nxt = [
                        bigp.tile([128, T_IN[2]], F32, tag=f"xT_{cj}")
                        for cj in range(4)
                    ]
                    for mi, (r0, nr) in enumerate(((0, 4), (4, 4), (8, 1))):
                        mtok = nr * wl1
                        for cj in range(4):
                            tp = pstp.tile([128, 128], F32, tag="tr")
                            nc.tensor.transpose(
                                tp[:, :mtok],
                                x2tok[mi][:mtok, cj * 128 : (cj + 1) * 128],
                                ident[:mtok, :mtok],
                            )
                            nc.any.tensor_copy(
                                nxt[cj][:, mi * 128 : mi * 128 + mtok], tp[:, :mtok]
                            )
                    xT = nxt
                else:
                    # final merge: token-major [64, 1024], straight to DRAM.
                    # wm2 (8 MB) is streamed; k outer so each chunk loads once.
                    nk = 4 * d // 128  # 16
                    fstg = constp.tile([T_MG[2], d2], F32, tag="fin", name="fstg")
                    ps0 = psp.tile([T_MG[2], 512], F32, tag="mm")
                    ps1 = psp.tile([T_MG[2], 512], F32, tag="yp")
                    for k in range(nk):
                        g, ci = divmod(k, nct)
                        wmk = wm2p.tile([128, d2], F32R, tag="wm2")
                        nc.gpsimd.dma_start(
                            out=wmk[:], in_=wm_d[2][k * 128 : (k + 1) * 128, :]
                        )
                        cg = blkp.tile([128, T_MG[2]], BF16, tag="mgl2", name="mgl2")
                        nc.vector.tensor_copy(
                            cg[:].rearrange("c (r w) -> c r w", r=4),
                            gview[g][ci][:],
                        )
                        nc.tensor.matmul(
                            ps0[:],
                            cg[:],
                            wmk[:, :512],
                            start=(k == 0),
                            stop=(k == nk - 1),
                        )
                        nc.tensor.matmul(
                            ps1[:],
                            cg[:],
                            wmk[:, 512:],
                            start=(k == 0),
                            stop=(k == nk - 1),
                        )
                    nc.any.tensor_copy(fstg[:, :512], ps0[:])
                    nc.any.tensor_copy(fstg[:, 512:], ps1[:])
                    nc.sync.dma_start(out=fin_d[:], in_=fstg[:])

    nc.compile()
    return nc


_NC_CACHE = {}


def _get_nc():
    if "nc" not in _NC_CACHE:
        _NC_CACHE["nc"] = _build_program()
    return _NC_CACHE["nc"]


def _host_prep(params):
    """Per-level host-side weight folding (cheap, O(d^3) numpy)."""
    prep = []
    kk = np.arange(WD)[None, :] - np.arange(TB)[:, None]  # j - i
    band = (kk >= 0) & (kk < K)
    for l in range(NLEV):
        p = params[l]
        wq = np.asarray(p["wq"], np.float32)
        wk = np.asarray(p["wk"], np.float32)
        wv = np.asarray(p["wv"], np.float32)
        wo = np.asarray(p["wo"], np.float32)
        bq = np.asarray(p["bq"], np.float32)
        bv = np.asarray(p["bv"], np.float32)
        bo = np.asarray(p["bo"], np.float32)
        rb = np.asarray(p["rel_bias"], np.float32)[0]  # [K]
        wqk = (wq @ wk.T).astype(np.float32)
        bqk = (wk @ bq).astype(np.float32)
        wvo = (wv @ wo).astype(np.float32)
        bvo = (bv @ wo + bo).astype(np.float32)
        mexp = np.zeros((TB, WD), np.float32)
        mexp[band] = np.exp(rb / np.sqrt(np.float32(D[l])))[kk[band]]
        mexp = np.tile(mexp, (1, 4))
        wm = np.asarray(p["w_merge"], np.float32)
        if l == 2:
            import ml_dtypes

            wm = wm.astype(ml_dtypes.bfloat16)
        prep.append((wqk, bqk, wvo, bvo, mexp, wm))
    return prep


def kernel(x, params):
    from concourse.bass_utils import run_bass_kernel_spmd

    x = np.asarray(x, np.float32)
    prep = _host_prep(params)
    nc = _get_nc()

    in_maps = []
    for core in range(8):
        b, s = divmod(core, 4)
        rows = np.arange(OWN_ROWS * s, OWN_ROWS * s + ROWS_IN[0]) % GRID
        x_sh = x[b].reshape(GRID, GRID, C0)[rows].reshape(T_IN[0], C0)
        m = {"x": np.ascontiguousarray(x_sh)}
        for l in range(NLEV):
            wqk, bqk, wvo, bvo, mexp, wm = prep[l]
            m[f"wqk{l}"] = wqk
            m[f"bqk{l}"] = bqk
            m[f"wvo{l}"] = wvo
            m[f"bvo{l}"] = bvo
            m[f"mexp{l}"] = mexp
            m[f"wm{l}"] = wm
        in_maps.append(m)

    res = run_bass_kernel_spmd(nc, in_maps, list(range(8)))

    skips = [
        np.empty((B, GRID * GRID // 4**l, D[l]), np.float32) for l in range(NLEV)
    ]
    fin = np.empty((B, 256, 1024), np.float32)
    for core in range(8):
        b, s = divmod(core, 4)
        r = res.results[core]
        for l in range(NLEV):
            ts = T_SKIP[l]
            skips[l][b, s * ts : (s + 1) * ts] = r[f"skip{l}"].T
        fin[b, s * 64 : (s + 1) * 64] = r["fin"]
    return (fin, skips[0], skips[1], skips[2])
